# revision 13
# baseline (speedup 1.0000x reference)
"""Trainium2 Bass kernel for nn_CliffordSteerableKernel (Cl(3,0), 7^3 grid).

Sharding (8 NeuronCores, SPMD — identical program, per-core weight slices):
  - Layers 1-3 (64ch), layer-4 linear + silu (1024ch): replicated per core.
    Layout: channels on partitions, blades x n(343->344) along the free dim.
  - Layer-4 sgp wl/wr matmuls (1024x1024 per grade), normalize, geometric
    product, LayerNorm partials, shell and output: sharded by c_out
    (128 channels / 4 c_out rows per core).
  - LayerNorm channel-mean: AllReduce of a single (1,344) row.
  - Final weighted-Cayley einsum == gather+scale: for each (l,m) output blade
    pair exactly one input blade k* = blade(mask_l ^ mask_m) contributes.
"""
import itertools
import math
import os
from contextlib import ExitStack

import numpy as np

DIM = 3
NB = 8
C_IN = 32
C_OUT = 32
K = 7
N = K ** DIM            # 343
NL = N + 1              # 344 (even, f32r-compatible)
HID = 64
FACTOR = 20.0 / K ** (DIM - 1)
SQ_EPS = 1e-12
EPS = 1e-6
RT2 = math.sqrt(2.0)

MM_DT_FLAG = os.environ.get("CSK_MM_DT", "f32")   # "f32" | "f32r"
DEBUG_OUTS = bool(int(os.environ.get("CSK_DEBUG", "0")))
N_CORES = 8
CSH = 1024 // N_CORES   # 128

GRADES = np.array([0, 1, 1, 1, 2, 2, 2, 3])
BLADES_OF = {0: [0], 1: [1, 2, 3], 2: [4, 5, 6], 3: [7]}
MASK = [0, 1, 2, 4, 3, 5, 6, 7]
M2B = {m: i for i, m in enumerate(MASK)}


def _build_algebra():
    blades = [c for g in range(DIM + 1) for c in itertools.combinations(range(DIM), g)]
    masks = [sum(1 << i for i in b) for b in blades]
    idx = {m: i for i, m in enumerate(masks)}
    cayley = np.zeros((NB,) * 3, np.float32)
    for i, ma in enumerate(masks):
        for kk, mb in enumerate(masks):
            s, t = 0, ma >> 1
            while t:
                s += bin(t & mb).count("1")
                t >>= 1
            sign = -1.0 if (s & 1) else 1.0
            cayley[i, idx[ma ^ mb], kk] = sign
    starts = np.concatenate(
        [[0], np.cumsum([math.comb(DIM, g) for g in range(DIM + 1)])]).astype(int)
    paths = np.zeros((DIM + 1,) * 3, bool)
    for gi in range(DIM + 1):
        for gj in range(DIM + 1):
            for gk in range(DIM + 1):
                blk = cayley[starts[gi]:starts[gi + 1], starts[gj]:starts[gj + 1],
                             starts[gk]:starts[gk + 1]]
                paths[gi, gj, gk] = bool(np.any(blk != 0))
    path_idx = np.argwhere(paths)
    return cayley, {tuple(p): i for i, p in enumerate(path_idx)}


CAYLEY, PATH_LUT = _build_algebra()


def _gp_weight_cols(gpw):
    """(C,20) -> (C,64): m[c, j*8+i] = CAYLEY[i,j,k] * gpw[c, path(gi,gj,gk)],
    k = blade(mask_i ^ mask_j)."""
    C = gpw.shape[0]
    out = np.zeros((C, 64), np.float32)
    for j in range(NB):
        for i in range(NB):
            kk = M2B[MASK[i] ^ MASK[j]]
            out[:, j * 8 + i] = CAYLEY[i, j, kk] * gpw[:, PATH_LUT[(GRADES[i], GRADES[j], GRADES[kk])]]
    return out


def _wlm_cols(cw):
    out = np.zeros((C_OUT * C_IN, 64), np.float32)
    for l in range(NB):
        for m in range(NB):
            ks = M2B[MASK[l] ^ MASK[m]]
            sign = CAYLEY[ks, l, m]
            p = PATH_LUT[(GRADES[ks], GRADES[l], GRADES[m])]
            out[:, l * 8 + m] = (sign * cw[:, :, p]).reshape(-1)
    return out


def _host_prep(params):
    g = lambda a: np.asarray(a, np.float32)
    layers = params["layers"]

    axes = [np.arange(K, dtype=np.float32) for _ in range(DIM)]
    grid = np.stack(np.meshgrid(*axes, indexing="ij"), -1) - K // 2
    rel = grid.reshape(-1, DIM) / max(K // 2, 1.0)
    qpos = (rel ** 2).sum(-1)
    sig = float(g(params["rel_pos_sigma"]).reshape(()))
    x0 = np.zeros((NB, NL), np.float32)
    x0[0, :N] = np.exp(-qpos / sig ** 2)
    x0[1:4, :N] = rel.T
    qpos_row = np.zeros((1, NL), np.float32)
    qpos_row[0, :N] = qpos

    shared = {"x0": x0, "qpos": qpos_row}

    def percol(co, lay):
        a = g(lay["silu_a"]).reshape(co, 4)
        b = g(lay["silu_b"]).reshape(co, 4)
        blin = g(lay["b"]).reshape(co)
        sna = 1.0 / (1.0 + np.exp(-g(lay["sgp_na"]).reshape(co, 4)))
        col = np.zeros((co, 84), np.float32)
        col[:, 0:4] = a
        col[:, 4:8] = b
        col[:, 4] = a[:, 0] * blin + b[:, 0]
        col[:, 8] = blin
        col[:, 9:13] = sna
        col[:, 13:17] = 1.0 - sna + EPS
        col[:, 17] = g(lay["ln_a"]).reshape(co)
        col[:, 18:82] = _gp_weight_cols(g(lay["sgp_gpw"]))
        col[:, 82] = g(lay["sgp_bl"]).reshape(co)
        col[:, 83] = 0.0
        return col

    for L in range(3):
        lay = layers[L]
        ci = 1 if L == 0 else HID
        w, wl, wr = g(lay["w"]), g(lay["sgp_wl"]), g(lay["sgp_wr"])
        wpack = np.zeros((ci, 4 * HID), np.float32)
        wlr = np.zeros((HID, 4 * 2 * HID), np.float32)
        for gr in range(4):
            wpack[:, gr * HID:(gr + 1) * HID] = w[:, :, gr].T
            wlr[:, gr * 128:gr * 128 + HID] = wl[:, :, gr].T
            wlr[:, gr * 128 + HID:(gr + 1) * 128] = wr[:, :, gr].T
        shared[f"l{L}_w"] = wpack
        shared[f"l{L}_wlr"] = wlr
        shared[f"l{L}_col"] = percol(HID, lay)

    l4 = layers[3]
    w4 = g(l4["w"])
    w4T = np.zeros((HID, 4 * 1024), np.float32)
    for gr in range(4):
        w4T[:, gr * 1024:(gr + 1) * 1024] = w4[:, :, gr].T
    shared["w4T"] = w4T
    a4 = g(l4["silu_a"]).reshape(1024, 4)
    b4 = g(l4["silu_b"]).reshape(1024, 4)
    blin4 = g(l4["b"]).reshape(1024)
    w4col = np.zeros((1024, 9), np.float32)
    w4col[:, 0:4] = a4
    w4col[:, 4:8] = b4
    w4col[:, 4] = a4[:, 0] * blin4 + b4[:, 0]
    w4col[:, 8] = blin4
    shared["w4col"] = w4col

    wl4, wr4 = g(l4["sgp_wl"]), g(l4["sgp_wr"])
    sna4 = 1.0 / (1.0 + np.exp(-g(l4["sgp_na"]).reshape(1024, 4)))
    bl4 = g(l4["sgp_bl"]).reshape(1024)
    lna4 = g(l4["ln_a"]).reshape(1024)
    mw4 = _gp_weight_cols(g(l4["sgp_gpw"]))
    wlm = _wlm_cols(g(params["cayley_weight"]))
    ssig = g(params["shell_sigma"]).reshape(1024, NB)

    in_maps = []
    for d in range(N_CORES):
        sl = slice(d * CSH, (d + 1) * CSH)
        wlr4 = np.zeros((2, 4, 8, 128, 128), np.float32)
        for li, wmat in enumerate((wl4, wr4)):
            for gr in range(4):
                for kt in range(8):
                    wlr4[li, gr, kt] = wmat[sl, kt * 128:(kt + 1) * 128, gr].T
        w4myT = np.zeros((HID, 4 * CSH), np.float32)
        for gr in range(4):
            w4myT[:, gr * CSH:(gr + 1) * CSH] = w4[sl, :, gr].T
        col = np.zeros((CSH, 154), np.float32)
        col[:, 0:4] = a4[sl]
        col[:, 4:8] = b4[sl]
        col[:, 4] = a4[sl, 0] * blin4[sl] + b4[sl, 0]
        col[:, 8] = blin4[sl]
        col[:, 9:13] = sna4[sl]
        col[:, 13:17] = 1.0 - sna4[sl] + EPS
        col[:, 17] = lna4[sl] * FACTOR
        col[:, 18:82] = mw4[sl]
        col[:, 82:90] = -ssig[sl]
        col[:, 90:154] = wlm[sl]
        m = dict(shared)
        m["wlr4"] = wlr4
        m["w4myT"] = w4myT
        m["l4col"] = col
        m["bl4"] = np.stack([bl4[sl], np.zeros(CSH, np.float32)], axis=1)
        in_maps.append(m)
    return in_maps


_PROG_CACHE = {}


def _build_program(mm_dt_flag=MM_DT_FLAG, no_collectives=False, debug=DEBUG_OUTS):
    key = (mm_dt_flag, no_collectives, debug)
    if key in _PROG_CACHE:
        return _PROG_CACHE[key]

    import concourse.bass_isa as bass_isa
    import concourse.mybir as mybir
    import concourse.tile as tile
    from concourse import bacc

    F32 = mybir.dt.float32
    MMD = mybir.dt.float32r if mm_dt_flag == "f32r" else F32
    AF = mybir.ActivationFunctionType
    OP = mybir.AluOpType

    nc = bacc.Bacc("TRN2", target_bir_lowering=False, debug=False,
                   num_devices=N_CORES)

    def dri(name, shape, dt=F32):
        return nc.dram_tensor(name, list(shape), dt, kind="ExternalInput")

    x0_d = dri("x0", (NB, NL))
    qpos_d = dri("qpos", (1, NL))
    lw_d = [dri(f"l{L}_w", ((1 if L == 0 else HID), 4 * HID)) for L in range(3)]
    lwlr_d = [dri(f"l{L}_wlr", (HID, 4 * 2 * HID)) for L in range(3)]
    lcol_d = [dri(f"l{L}_col", (HID, 84)) for L in range(3)]
    w4T_d = dri("w4T", (HID, 4 * 1024), MMD)
    w4col_d = dri("w4col", (1024, 9))
    wlr4_d = dri("wlr4", (2, 4, 8, 128, 128), MMD)
    w4myT_d = dri("w4myT", (HID, 4 * CSH), MMD)
    l4col_d = dri("l4col", (CSH, 154))
    bl4_d = dri("bl4", (CSH, 2))
    out_d = nc.dram_tensor("out", [C_OUT // N_CORES * NB, C_IN * NB, N], F32,
                           kind="ExternalOutput")
    ar_in = nc.dram_tensor("ar_in", [1, NL], F32)
    ar_out = nc.dram_tensor("ar_out", [1, NL], F32)
    for v in (SQ_EPS, 2 * SQ_EPS):
        t = nc.alloc_sbuf_tensor(f"const-eps-{v}", [128, 1], F32)
        nc.gpsimd.memset(t.ap(), v)
        nc.const_aps.aps[(F32, v)] = t.ap()
    nc.all_engine_barrier()

    dbg = {}
    if debug:
        for nm, shape in [("d_x1", (HID, NB * NL)), ("d_x3", (HID, NB * NL)),
                          ("d_x4s", (128, NB * NL)), ("d_xr", (128, NB * NL)),
                          ("d_xo", (128, NB * NL)), ("d_sum", (1, NL)),
                          ("d_T", (128, NB * NL))]:
            dbg[nm] = nc.dram_tensor(nm, list(shape), F32, kind="ExternalOutput")

    with tile.TileContext(nc) as tc, ExitStack() as ctx:
        cst = ctx.enter_context(tc.tile_pool(name="cst", bufs=1))
        big = ctx.enter_context(tc.tile_pool(name="big", bufs=1))
        wk = ctx.enter_context(tc.tile_pool(name="wk", bufs=1))
        wst = ctx.enter_context(tc.tile_pool(name="wst", bufs=6))
        ost = ctx.enter_context(tc.tile_pool(name="ost", bufs=2))
        ps = ctx.enter_context(tc.tile_pool(name="ps", bufs=8, space="PSUM"))

        def load(pool, dram, dt=F32, tag=None):
            t = pool.tile(list(dram.shape), dt, tag=tag or f"cst_{dram.name}",
                          name=f"ld_{dram.name}")
            nc.sync.dma_start(t[:], dram.ap())
            return t

        qpos_t = load(cst, qpos_d)
        lcol_t = [load(cst, d) for d in lcol_d]
        w4col_t = []
        for ct in range(8):
            t = cst.tile([128, 9], mybir.dt.float32, tag=f"w4col{ct}")
            nc.sync.dma_start(t[:], w4col_d.ap()[ct * 128:(ct + 1) * 128, :])
            w4col_t.append(t)
        w4myT_t = load(cst, w4myT_d, dt=MMD)
        l4col_t = load(cst, l4col_d)
        bl4_t = load(cst, bl4_d)

        def bsl(t, b, nb=1, P=None):
            a = t[:, b * NL:(b + nb) * NL] if P is None else t[:P, b * NL:(b + nb) * NL]
            return a

        # ---------- silu on psum blade tiles -> outt (co, 8NL) ----------
        def silu_block(co, yps, colt, outt):
            sqA = wk.tile([128, NB * NL], F32, tag="big8")
            for b in range(1, NB):
                nc.scalar.activation(sqA[:co, (b - 1) * NL:b * NL], yps[b], AF.Square)
            qt = wk.tile([128, 3 * NL], F32, tag="qt")
            nc.vector.tensor_tensor(qt[:co, 0:NL], sqA[:co, 0:NL], sqA[:co, NL:2 * NL], op=OP.add)
            nc.vector.tensor_tensor(qt[:co, 0:NL], qt[:co, 0:NL], sqA[:co, 2 * NL:3 * NL], op=OP.add)
            nc.gpsimd.tensor_tensor(qt[:co, NL:2 * NL], sqA[:co, 3 * NL:4 * NL], sqA[:co, 4 * NL:5 * NL], op=OP.add)
            nc.gpsimd.tensor_tensor(qt[:co, NL:2 * NL], qt[:co, NL:2 * NL], sqA[:co, 5 * NL:6 * NL], op=OP.add)
            nc.gpsimd.tensor_copy(qt[:co, 2 * NL:], sqA[:co, 6 * NL:7 * NL])
            nc.scalar.activation(qt[:co], qt[:co], AF.Sqrt, bias=SQ_EPS)  # in-place -> rt
            gt = wk.tile([128, 4 * NL], F32, tag="gt")
            nc.scalar.activation(gt[:co, 0:NL], yps[0], AF.Sigmoid,
                                 scale=colt[:, 0:1], bias=colt[:, 4:5])
            for gr in range(1, 4):
                nc.scalar.activation(gt[:co, gr * NL:(gr + 1) * NL],
                                     qt[:co, (gr - 1) * NL:gr * NL], AF.Sigmoid,
                                     scale=colt[:, gr:gr + 1], bias=colt[:, 4 + gr:5 + gr])
            nc.vector.scalar_tensor_tensor(bsl(outt, 0, P=co), yps[0], colt[:, 8:9],
                                           gt[:co, 0:NL], op0=OP.add, op1=OP.mult)
            for b in range(1, NB):
                nc.vector.tensor_tensor(bsl(outt, b, P=co), yps[b],
                                        gt[:co, GRADES[b] * NL:(GRADES[b] + 1) * NL],
                                        op=OP.mult)

        # ---------- normalize wr-psum tiles -> xrt ----------
        def normalize_block(co, rps, colt, xrt):
            sqA = wk.tile([128, NB * NL], F32, tag="big8")
            nq = wk.tile([128, 4 * NL], F32, tag="nq")
            nc.scalar.activation(nq[:co, 0:NL], rps[0], AF.Square)
            for b in range(1, NB):
                nc.scalar.activation(sqA[:co, (b - 1) * NL:b * NL], rps[b], AF.Square)
            nc.vector.tensor_tensor(nq[:co, NL:2 * NL], sqA[:co, 0:NL], sqA[:co, NL:2 * NL], op=OP.add)
            nc.vector.tensor_tensor(nq[:co, NL:2 * NL], nq[:co, NL:2 * NL], sqA[:co, 2 * NL:3 * NL], op=OP.add)
            nc.gpsimd.tensor_tensor(nq[:co, 2 * NL:3 * NL], sqA[:co, 3 * NL:4 * NL], sqA[:co, 4 * NL:5 * NL], op=OP.add)
            nc.gpsimd.tensor_tensor(nq[:co, 2 * NL:3 * NL], nq[:co, 2 * NL:3 * NL], sqA[:co, 5 * NL:6 * NL], op=OP.add)
            nc.gpsimd.tensor_copy(nq[:co, 3 * NL:], sqA[:co, 6 * NL:7 * NL])
            nc.scalar.activation(nq[:co], nq[:co], AF.Sqrt, bias=SQ_EPS)
            for gr in range(4):
                nc.vector.tensor_scalar(nq[:co, gr * NL:(gr + 1) * NL],
                                        nq[:co, gr * NL:(gr + 1) * NL],
                                        colt[:, 9 + gr:10 + gr], colt[:, 13 + gr:14 + gr],
                                        op0=OP.mult, op1=OP.add)
            rec = wk.tile([128, 4 * NL], F32, tag="gt")
            scr = wk.tile([128, 4 * NL], F32, tag="qt")
            nc.vector.reciprocal_approx_accurate(rec[:co], nq[:co], scr[:co])
            for b in range(NB):
                nc.vector.tensor_tensor(bsl(xrt, b, P=co), rps[b],
                                        rec[:co, GRADES[b] * NL:(GRADES[b] + 1) * NL],
                                        op=OP.mult)

        # ---------- gp + combine with wl psum + bias ----------
        def gp_block(co, xs_t, xrt, lps, colt, blcol, xot):
            for j in range(NB):
                pb = wk.tile([128, NB * NL], F32, tag="big8")
                for i in range(NB):
                    kk = M2B[MASK[i] ^ MASK[j]]
                    eng = nc.vector
                    eng.scalar_tensor_tensor(bsl(pb, i, P=co), bsl(xs_t, i, P=co),
                                             colt[:, 18 + j * 8 + i:19 + j * 8 + i],
                                             bsl(xrt, kk, P=co), op0=OP.mult, op1=OP.mult)
                eng = nc.gpsimd
                eng.tensor_tensor(pb[:co, :4 * NL], pb[:co, :4 * NL], pb[:co, 4 * NL:], op=OP.add)
                eng.tensor_tensor(pb[:co, :2 * NL], pb[:co, :2 * NL], pb[:co, 2 * NL:4 * NL], op=OP.add)
                eng.tensor_tensor(pb[:co, :NL], pb[:co, :NL], pb[:co, NL:2 * NL], op=OP.add)
                bc = blcol[:, 0:1] if j == 0 else blcol[:, 1:2]
                nc.vector.scalar_tensor_tensor(bsl(xot, j, P=co), lps[j], bc,
                                               pb[:co, :NL], op0=OP.add, op1=OP.add)

        # ---------- LayerNorm channel-sum of norms ----------
        def ln_norm_sums(co, xot):
            sqo = wk.tile([128, NB * NL], F32, tag="big8")
            nc.scalar.activation(sqo[:co], xot[:co], AF.Square)
            nc.vector.tensor_tensor(sqo[:co, :4 * NL], sqo[:co, :4 * NL], sqo[:co, 4 * NL:], op=OP.add)
            nc.vector.tensor_tensor(sqo[:co, :2 * NL], sqo[:co, :2 * NL], sqo[:co, 2 * NL:4 * NL], op=OP.add)
            nc.vector.tensor_tensor(sqo[:co, :NL], sqo[:co, :NL], sqo[:co, NL:2 * NL], op=OP.add)
            nc.scalar.activation(sqo[:co, :NL], sqo[:co, :NL], AF.Sqrt, bias=2 * SQ_EPS)
            csum = wk.tile([128, NL], F32, tag="csum")
            nc.gpsimd.partition_all_reduce(csum[:co], sqo[:co, :NL], channels=co,
                                           reduce_op=bass_isa.ReduceOp.add)
            return csum

        def recip_row(row, co, nch):
            rr = wk.tile([128, NL], F32, tag="rr")
            nc.vector.tensor_scalar(rr[:co], row[:co], 1.0 / nch, RT2 * EPS,
                                    op0=OP.mult, op1=OP.add)
            rro = wk.tile([128, NL], F32, tag="rro")
            rrs = wk.tile([128, NL], F32, tag="rrs")
            nc.vector.reciprocal_approx_accurate(rro[:co], rr[:co], rrs[:co])
            return rro

        # ================= layers 1..3 =================
        x_cur = None
        for L in range(3):
            colt = lcol_t[L]
            lw_t = wst.tile([(1 if L == 0 else HID), 4 * HID], F32, tag="lww", bufs=2)
            nc.sync.dma_start(lw_t[:], lw_d[L].ap())
            lwlr_t = wst.tile([HID, 4 * 2 * HID], F32, tag="lwlr", bufs=2)
            nc.sync.dma_start(lwlr_t[:], lwlr_d[L].ap())
            yps = []
            for b in range(NB):
                gr = GRADES[b]
                pt = ps.tile([128, 512], F32, tag="ps")
                if L == 0:
                    x0b = wst.tile([1, NL], F32, tag="x0s", bufs=2)
                    nc.sync.dma_start(x0b[:], x0_d.ap()[b:b + 1, :])
                    rhs = x0b[:]
                else:
                    rhs = bsl(x_cur, b, P=HID)
                nc.tensor.matmul(pt[:HID, :NL], lhsT=lw_t[:, gr * HID:(gr + 1) * HID],
                                 rhs=rhs, start=True, stop=True)
                yps.append(pt[:HID, :NL])
            xs = big.tile([128, NB * NL], F32, tag="pk1")
            silu_block(HID, yps, colt, xs)
            if debug and L == 0:
                nc.sync.dma_start(dbg["d_x1"].ap(), xs[:HID])
            wps = []
            for b in range(NB):
                gr = GRADES[b]
                pt = ps.tile([128, 512], F32, tag="ps")
                nc.tensor.matmul(pt[:, :NL], lhsT=lwlr_t[:, gr * 128:(gr + 1) * 128],
                                 rhs=bsl(xs, b, P=HID), start=True, stop=True)
                wps.append(pt)
            lps = [p[0:HID, :NL] for p in wps]
            rps = [p[HID:2 * HID, :NL] for p in wps]
            xr = big.tile([128, NB * NL], F32, tag="pk2")
            normalize_block(HID, rps, colt, xr)
            xo = big.tile([128, NB * NL], F32, tag="pk3")
            gp_block(HID, xs, xr, lps, colt, colt[:, 82:84], xo)
            csum = ln_norm_sums(HID, xo)
            rro = recip_row(csum, HID, HID)
            xn = big.tile([128, NB * NL], MMD if L == 2 else F32, tag="pk4")
            for b in range(NB):
                nc.vector.scalar_tensor_tensor(bsl(xn, b, P=HID), bsl(xo, b, P=HID),
                                               colt[:, 17:18], rro[:HID, :NL],
                                               op0=OP.mult, op1=OP.mult)
            x_cur = xn
        x3r = x_cur
        if debug:
            nc.sync.dma_start(dbg["d_x3"].ap(), x3r[:HID].bitcast(F32))

        qb = cst.tile([128, NL], F32)
        nc.gpsimd.partition_broadcast(qb[:], qpos_t[:])

        # ================= layer 4 linear + silu (1024ch) =================
        x4s = []
        for ct in range(8):
            w4w = []
            for gr in range(4):
                wt = wst.tile([HID, 128], MMD, tag="w4w", bufs=4)
                nc.sync.dma_start(
                    wt[:], w4T_d.ap()[:, gr * 1024 + ct * 128:gr * 1024 + (ct + 1) * 128])
                w4w.append(wt)
            ypsl = []
            for b in range(NB):
                gr = GRADES[b]
                pt = ps.tile([128, 512], F32, tag="ps")
                nc.tensor.matmul(pt[:, :NL], lhsT=w4w[gr][:],
                                 rhs=bsl(x3r, b, P=HID), start=True, stop=True)
                ypsl.append(pt[:, :NL])
            xst = big.tile([128, NB * NL], MMD, tag=f"x4s{ct}")
            silu_block(128, ypsl, w4col_t[ct], xst)
            x4s.append(xst)

        ypsl = []
        for b in range(NB):
            gr = GRADES[b]
            pt = ps.tile([128, 512], F32, tag="ps")
            nc.tensor.matmul(pt[:, :NL], lhsT=w4myT_t[:, gr * CSH:(gr + 1) * CSH],
                             rhs=bsl(x3r, b, P=HID), start=True, stop=True)
            ypsl.append(pt[:, :NL])
        xmy = big.tile([128, NB * NL], F32, tag="pk1")
        silu_block(128, ypsl, l4col_t, xmy)
        if debug:
            nc.sync.dma_start(dbg["d_x4s"].ap(), xmy[:])

        # ================= layer 4 sgp (c-shard) =================
        def lin4(li):
            pts = [ps.tile([128, 512], F32, tag="ps", name=f"lin4_{li}_{b}")
                   for b in range(NB)]
            for kt in range(8):
                for gr in range(4):
                    wt = wst.tile([128, 128], MMD, tag="wt")
                    nc.sync.dma_start(wt[:], wlr4_d.ap()[li, gr, kt])
                    for b in BLADES_OF[gr]:
                        nc.tensor.matmul(pts[b][:, :NL], lhsT=wt[:],
                                         rhs=bsl(x4s[kt], b),
                                         start=(kt == 0), stop=(kt == 7))
            return pts

        rpts = lin4(1)
        xr4 = big.tile([128, NB * NL], F32, tag="pk2")
        normalize_block(128, [p[:, :NL] for p in rpts], l4col_t, xr4)
        if debug:
            nc.sync.dma_start(dbg["d_xr"].ap(), xr4[:])
        lpts = lin4(0)
        xo4 = big.tile([128, NB * NL], F32, tag="pk3")
        gp_block(128, xmy, xr4, [p[:, :NL] for p in lpts], l4col_t, bl4_t, xo4)
        if debug:
            nc.sync.dma_start(dbg["d_xo"].ap(), xo4[:])

        csum4 = ln_norm_sums(128, xo4)
        if no_collectives:
            sum_b = wk.tile([128, NL], F32, tag="csum")
            nc.vector.tensor_scalar_mul(sum_b[:], csum4[:], 8.0)
        else:
            nc.sync.dma_start(ar_in.ap(), csum4[0:1, :])
            nc.gpsimd.collective_compute(
                "AllReduce", OP.add, replica_groups=[list(range(N_CORES))],
                ins=[ar_in.ap().opt()], outs=[ar_out.ap().opt()])
            sum_t = wk.tile([1, NL], F32, tag="sumt")
            nc.sync.dma_start(sum_t[:], ar_out.ap())
            sum_b = wk.tile([128, NL], F32, tag="csum")
            nc.gpsimd.partition_broadcast(sum_b[:], sum_t[:])
        if debug:
            nc.sync.dma_start(dbg["d_sum"].ap(), sum_b[0:1, :])
        rb4 = recip_row(sum_b, 128, 1024)

        Gt = big.tile([128, NB * NL], F32, tag="x4s0")
        for b in range(NB):
            nc.scalar.activation(bsl(Gt, b), qb[:], AF.Exp,
                                 scale=l4col_t[:, 82 + b:83 + b])
        for b in range(NB):
            eng = nc.vector if b % 2 == 0 else nc.gpsimd
            eng.tensor_tensor(bsl(Gt, b), bsl(Gt, b), rb4[:, :NL], op=OP.mult)
        for b in range(NB):
            nc.vector.scalar_tensor_tensor(bsl(xo4, b), bsl(xo4, b),
                                           l4col_t[:, 17:18], bsl(Gt, b),
                                           op0=OP.mult, op1=OP.mult)
        if debug:
            nc.sync.dma_start(dbg["d_T"].ap(), xo4[:])

        out_v = out_d.ap().rearrange("(ol l) (i m) n -> ol l i m n", l=NB, m=NB)
        for l in range(NB):
            for m in range(NB):
                ks = M2B[MASK[l] ^ MASK[m]]
                ot = ost.tile([128, NL], F32, tag="ot")
                eng = nc.vector if (l * 8 + m) % 2 == 0 else nc.gpsimd
                eng.tensor_scalar_mul(ot[:], bsl(xo4, ks),
                                      l4col_t[:, 90 + l * 8 + m:91 + l * 8 + m])
                nc.sync.dma_start(out_v[:, l, :, m, :], ot[:, :N])

    nc.compile()
    _PROG_CACHE[key] = nc
    return nc


def kernel(params):
    from concourse.bass_utils import run_bass_kernel_spmd
    in_maps = _host_prep(params)
    nc = _build_program()
    res = run_bass_kernel_spmd(nc, in_maps, core_ids=list(range(N_CORES)))
    out = np.concatenate([r["out"] for r in res.results], axis=0)
    return out.reshape(C_OUT * NB, C_IN * NB, K, K, K)


# revision 14
# speedup vs baseline: 76.6825x; 76.6825x over previous
"""Trainium2 Bass kernel for nn_CliffordSteerableKernel (Cl(3,0), 7^3 grid).

Sharding (8 NeuronCores, SPMD — identical program, per-core weight slices):
  - Layers 1-3 (64ch), layer-4 linear + silu (1024ch): replicated per core.
    Layout: channels on partitions, blades x n(343->344) along the free dim.
  - Layer-4 sgp wl/wr matmuls (1024x1024 per grade), normalize, geometric
    product, LayerNorm partials, shell and output: sharded by c_out
    (128 channels / 4 c_out rows per core).
  - LayerNorm channel-mean: AllReduce of a single (1,344) row.
  - Final weighted-Cayley einsum == gather+scale: for each (l,m) output blade
    pair exactly one input blade k* = blade(mask_l ^ mask_m) contributes.
"""
import itertools
import math
import os
from contextlib import ExitStack

import numpy as np

DIM = 3
NB = 8
C_IN = 32
C_OUT = 32
K = 7
N = K ** DIM            # 343
NL = N + 1              # 344 (even, f32r-compatible)
HID = 64
FACTOR = 20.0 / K ** (DIM - 1)
SQ_EPS = 1e-12
EPS = 1e-6
RT2 = math.sqrt(2.0)

MM_DT_FLAG = os.environ.get("CSK_MM_DT", "f32")   # "f32" | "f32r"
DEBUG_OUTS = bool(int(os.environ.get("CSK_DEBUG", "0")))
N_CORES = 8
CSH = 1024 // N_CORES   # 128

GRADES = np.array([0, 1, 1, 1, 2, 2, 2, 3])
BLADES_OF = {0: [0], 1: [1, 2, 3], 2: [4, 5, 6], 3: [7]}
MASK = [0, 1, 2, 4, 3, 5, 6, 7]
M2B = {m: i for i, m in enumerate(MASK)}


def _build_algebra():
    blades = [c for g in range(DIM + 1) for c in itertools.combinations(range(DIM), g)]
    masks = [sum(1 << i for i in b) for b in blades]
    idx = {m: i for i, m in enumerate(masks)}
    cayley = np.zeros((NB,) * 3, np.float32)
    for i, ma in enumerate(masks):
        for kk, mb in enumerate(masks):
            s, t = 0, ma >> 1
            while t:
                s += bin(t & mb).count("1")
                t >>= 1
            sign = -1.0 if (s & 1) else 1.0
            cayley[i, idx[ma ^ mb], kk] = sign
    starts = np.concatenate(
        [[0], np.cumsum([math.comb(DIM, g) for g in range(DIM + 1)])]).astype(int)
    paths = np.zeros((DIM + 1,) * 3, bool)
    for gi in range(DIM + 1):
        for gj in range(DIM + 1):
            for gk in range(DIM + 1):
                blk = cayley[starts[gi]:starts[gi + 1], starts[gj]:starts[gj + 1],
                             starts[gk]:starts[gk + 1]]
                paths[gi, gj, gk] = bool(np.any(blk != 0))
    path_idx = np.argwhere(paths)
    return cayley, {tuple(p): i for i, p in enumerate(path_idx)}


CAYLEY, PATH_LUT = _build_algebra()


def _gp_weight_cols(gpw):
    """(C,20) -> (C,64): m[c, j*8+i] = CAYLEY[i,j,k] * gpw[c, path(gi,gj,gk)],
    k = blade(mask_i ^ mask_j)."""
    C = gpw.shape[0]
    out = np.zeros((C, 64), np.float32)
    for j in range(NB):
        for i in range(NB):
            kk = M2B[MASK[i] ^ MASK[j]]
            out[:, j * 8 + i] = CAYLEY[i, j, kk] * gpw[:, PATH_LUT[(GRADES[i], GRADES[j], GRADES[kk])]]
    return out


def _wlm_cols(cw):
    out = np.zeros((C_OUT * C_IN, 64), np.float32)
    for l in range(NB):
        for m in range(NB):
            ks = M2B[MASK[l] ^ MASK[m]]
            sign = CAYLEY[ks, l, m]
            p = PATH_LUT[(GRADES[ks], GRADES[l], GRADES[m])]
            out[:, l * 8 + m] = (sign * cw[:, :, p]).reshape(-1)
    return out


def _host_prep(params):
    g = lambda a: np.asarray(a, np.float32)
    layers = params["layers"]

    axes = [np.arange(K, dtype=np.float32) for _ in range(DIM)]
    grid = np.stack(np.meshgrid(*axes, indexing="ij"), -1) - K // 2
    rel = grid.reshape(-1, DIM) / max(K // 2, 1.0)
    qpos = (rel ** 2).sum(-1)
    sig = float(g(params["rel_pos_sigma"]).reshape(()))
    x0 = np.zeros((NB, NL), np.float32)
    x0[0, :N] = np.exp(-qpos / sig ** 2)
    x0[1:4, :N] = rel.T
    qpos_row = np.zeros((1, NL), np.float32)
    qpos_row[0, :N] = qpos

    shared = {"x0": x0, "qpos": qpos_row}

    def percol(co, lay):
        a = g(lay["silu_a"]).reshape(co, 4)
        b = g(lay["silu_b"]).reshape(co, 4)
        blin = g(lay["b"]).reshape(co)
        sna = 1.0 / (1.0 + np.exp(-g(lay["sgp_na"]).reshape(co, 4)))
        col = np.zeros((co, 84), np.float32)
        col[:, 0:4] = a
        col[:, 4:8] = b
        col[:, 4] = a[:, 0] * blin + b[:, 0]
        col[:, 8] = blin
        col[:, 9:13] = sna
        col[:, 13:17] = 1.0 - sna + EPS
        col[:, 17] = g(lay["ln_a"]).reshape(co)
        col[:, 18:82] = _gp_weight_cols(g(lay["sgp_gpw"]))
        col[:, 82] = g(lay["sgp_bl"]).reshape(co)
        col[:, 83] = 0.0
        return col

    for L in range(3):
        lay = layers[L]
        ci = 1 if L == 0 else HID
        w, wl, wr = g(lay["w"]), g(lay["sgp_wl"]), g(lay["sgp_wr"])
        wpack = np.zeros((ci, 4 * HID), np.float32)
        wlr = np.zeros((HID, 4 * 2 * HID), np.float32)
        for gr in range(4):
            wpack[:, gr * HID:(gr + 1) * HID] = w[:, :, gr].T
            wlr[:, gr * 128:gr * 128 + HID] = wl[:, :, gr].T
            wlr[:, gr * 128 + HID:(gr + 1) * 128] = wr[:, :, gr].T
        shared[f"l{L}_w"] = wpack
        shared[f"l{L}_wlr"] = wlr
        shared[f"l{L}_col"] = percol(HID, lay)

    l4 = layers[3]
    w4 = g(l4["w"])
    w4T = np.zeros((HID, 4 * 1024), np.float32)
    for gr in range(4):
        w4T[:, gr * 1024:(gr + 1) * 1024] = w4[:, :, gr].T
    shared["w4T"] = w4T
    a4 = g(l4["silu_a"]).reshape(1024, 4)
    b4 = g(l4["silu_b"]).reshape(1024, 4)
    blin4 = g(l4["b"]).reshape(1024)
    w4col = np.zeros((1024, 9), np.float32)
    w4col[:, 0:4] = a4
    w4col[:, 4:8] = b4
    w4col[:, 4] = a4[:, 0] * blin4 + b4[:, 0]
    w4col[:, 8] = blin4
    shared["w4col"] = w4col

    wl4, wr4 = g(l4["sgp_wl"]), g(l4["sgp_wr"])
    sna4 = 1.0 / (1.0 + np.exp(-g(l4["sgp_na"]).reshape(1024, 4)))
    bl4 = g(l4["sgp_bl"]).reshape(1024)
    lna4 = g(l4["ln_a"]).reshape(1024)
    mw4 = _gp_weight_cols(g(l4["sgp_gpw"]))
    wlm = _wlm_cols(g(params["cayley_weight"]))
    ssig = g(params["shell_sigma"]).reshape(1024, NB)

    in_maps = []
    for d in range(N_CORES):
        sl = slice(d * CSH, (d + 1) * CSH)
        wlr4 = np.zeros((2, 4, 8, 128, 128), np.float32)
        for li, wmat in enumerate((wl4, wr4)):
            for gr in range(4):
                for kt in range(8):
                    wlr4[li, gr, kt] = wmat[sl, kt * 128:(kt + 1) * 128, gr].T
        w4myT = np.zeros((HID, 4 * CSH), np.float32)
        for gr in range(4):
            w4myT[:, gr * CSH:(gr + 1) * CSH] = w4[sl, :, gr].T
        col = np.zeros((CSH, 154), np.float32)
        col[:, 0:4] = a4[sl]
        col[:, 4:8] = b4[sl]
        col[:, 4] = a4[sl, 0] * blin4[sl] + b4[sl, 0]
        col[:, 8] = blin4[sl]
        col[:, 9:13] = sna4[sl]
        col[:, 13:17] = 1.0 - sna4[sl] + EPS
        col[:, 17] = lna4[sl] * FACTOR
        col[:, 18:82] = mw4[sl]
        col[:, 82:90] = -ssig[sl]
        col[:, 90:154] = wlm[sl]
        m = dict(shared)
        m["wlr4"] = wlr4
        m["w4myT"] = w4myT
        m["l4col"] = col
        m["bl4"] = np.stack([bl4[sl], np.zeros(CSH, np.float32)], axis=1)
        in_maps.append(m)
    return in_maps


_PROG_CACHE = {}


def _build_program(mm_dt_flag=MM_DT_FLAG, no_collectives=False, debug=DEBUG_OUTS):
    key = (mm_dt_flag, no_collectives, debug)
    if key in _PROG_CACHE:
        return _PROG_CACHE[key]

    import concourse.bass_isa as bass_isa
    import concourse.mybir as mybir
    import concourse.tile as tile
    from concourse import bacc

    F32 = mybir.dt.float32
    MMD = mybir.dt.float32r if mm_dt_flag == "f32r" else F32
    AF = mybir.ActivationFunctionType
    OP = mybir.AluOpType

    nc = bacc.Bacc("TRN2", target_bir_lowering=False, debug=False,
                   num_devices=N_CORES)

    def dri(name, shape, dt=F32):
        return nc.dram_tensor(name, list(shape), dt, kind="ExternalInput")

    x0_d = dri("x0", (NB, NL))
    qpos_d = dri("qpos", (1, NL))
    lw_d = [dri(f"l{L}_w", ((1 if L == 0 else HID), 4 * HID)) for L in range(3)]
    lwlr_d = [dri(f"l{L}_wlr", (HID, 4 * 2 * HID)) for L in range(3)]
    lcol_d = [dri(f"l{L}_col", (HID, 84)) for L in range(3)]
    w4T_d = dri("w4T", (HID, 4 * 1024), MMD)
    w4col_d = dri("w4col", (1024, 9))
    wlr4_d = dri("wlr4", (2, 4, 8, 128, 128), MMD)
    w4myT_d = dri("w4myT", (HID, 4 * CSH), MMD)
    l4col_d = dri("l4col", (CSH, 154))
    bl4_d = dri("bl4", (CSH, 2))
    out_d = nc.dram_tensor("out", [C_OUT // N_CORES * NB, C_IN * NB, N], F32,
                           kind="ExternalOutput")
    ar_in = nc.dram_tensor("ar_in", [1, NL], F32)
    ar_out = nc.dram_tensor("ar_out", [1, NL], F32)
    for v in (SQ_EPS, 2 * SQ_EPS):
        t = nc.alloc_sbuf_tensor(f"const-eps-{v}", [128, 1], F32)
        nc.gpsimd.memset(t.ap(), v)
        nc.const_aps.aps[(F32, v)] = t.ap()
    nc.all_engine_barrier()

    dbg = {}
    if debug:
        for nm, shape in [("d_x1", (HID, NB * NL)), ("d_x3", (HID, NB * NL)),
                          ("d_x4s", (128, NB * NL)), ("d_xr", (128, NB * NL)),
                          ("d_xo", (128, NB * NL)), ("d_sum", (1, NL)),
                          ("d_T", (128, NB * NL))]:
            dbg[nm] = nc.dram_tensor(nm, list(shape), F32, kind="ExternalOutput")

    with tile.TileContext(nc) as tc, ExitStack() as ctx:
        cst = ctx.enter_context(tc.tile_pool(name="cst", bufs=1))
        big = ctx.enter_context(tc.tile_pool(name="big", bufs=1))
        wk = ctx.enter_context(tc.tile_pool(name="wk", bufs=1))
        wst = ctx.enter_context(tc.tile_pool(name="wst", bufs=6))
        ost = ctx.enter_context(tc.tile_pool(name="ost", bufs=2))
        ps = ctx.enter_context(tc.tile_pool(name="ps", bufs=8, space="PSUM"))

        def load(pool, dram, dt=F32, tag=None):
            t = pool.tile(list(dram.shape), dt, tag=tag or f"cst_{dram.name}",
                          name=f"ld_{dram.name}")
            nc.sync.dma_start(t[:], dram.ap())
            return t

        qpos_t = load(cst, qpos_d)
        lcol_t = [load(cst, d) for d in lcol_d]
        w4col_t = []
        for ct in range(8):
            t = cst.tile([128, 9], mybir.dt.float32, tag=f"w4col{ct}")
            nc.sync.dma_start(t[:], w4col_d.ap()[ct * 128:(ct + 1) * 128, :])
            w4col_t.append(t)
        w4myT_t = load(cst, w4myT_d, dt=MMD)
        l4col_t = load(cst, l4col_d)
        bl4_t = load(cst, bl4_d)

        def bsl(t, b, nb=1, P=None):
            a = t[:, b * NL:(b + nb) * NL] if P is None else t[:P, b * NL:(b + nb) * NL]
            return a

        # ---------- silu on psum blade tiles -> outt (co, 8NL) ----------
        def silu_block(co, yps, colt, outt):
            sqA = wk.tile([128, NB * NL], F32, tag="big8")
            for b in range(1, NB):
                nc.scalar.activation(sqA[:co, (b - 1) * NL:b * NL], yps[b], AF.Square)
            qt = wk.tile([128, 3 * NL], F32, tag="qt")
            nc.vector.tensor_tensor(qt[:co, 0:NL], sqA[:co, 0:NL], sqA[:co, NL:2 * NL], op=OP.add)
            nc.vector.tensor_tensor(qt[:co, 0:NL], qt[:co, 0:NL], sqA[:co, 2 * NL:3 * NL], op=OP.add)
            nc.gpsimd.tensor_tensor(qt[:co, NL:2 * NL], sqA[:co, 3 * NL:4 * NL], sqA[:co, 4 * NL:5 * NL], op=OP.add)
            nc.gpsimd.tensor_tensor(qt[:co, NL:2 * NL], qt[:co, NL:2 * NL], sqA[:co, 5 * NL:6 * NL], op=OP.add)
            nc.gpsimd.tensor_copy(qt[:co, 2 * NL:], sqA[:co, 6 * NL:7 * NL])
            nc.scalar.activation(qt[:co], qt[:co], AF.Sqrt, bias=SQ_EPS)  # in-place -> rt
            gt = wk.tile([128, 4 * NL], F32, tag="gt")
            nc.scalar.activation(gt[:co, 0:NL], yps[0], AF.Sigmoid,
                                 scale=colt[:, 0:1], bias=colt[:, 4:5])
            for gr in range(1, 4):
                nc.scalar.activation(gt[:co, gr * NL:(gr + 1) * NL],
                                     qt[:co, (gr - 1) * NL:gr * NL], AF.Sigmoid,
                                     scale=colt[:, gr:gr + 1], bias=colt[:, 4 + gr:5 + gr])
            nc.vector.scalar_tensor_tensor(bsl(outt, 0, P=co), yps[0], colt[:, 8:9],
                                           gt[:co, 0:NL], op0=OP.add, op1=OP.mult)
            for b in range(1, NB):
                nc.vector.tensor_tensor(bsl(outt, b, P=co), yps[b],
                                        gt[:co, GRADES[b] * NL:(GRADES[b] + 1) * NL],
                                        op=OP.mult)

        # ---------- normalize wr-psum tiles -> xrt ----------
        def normalize_block(co, rps, colt, xrt):
            sqA = wk.tile([128, NB * NL], F32, tag="big8")
            nq = wk.tile([128, 4 * NL], F32, tag="nq")
            nc.scalar.activation(nq[:co, 0:NL], rps[0], AF.Square)
            for b in range(1, NB):
                nc.scalar.activation(sqA[:co, (b - 1) * NL:b * NL], rps[b], AF.Square)
            nc.vector.tensor_tensor(nq[:co, NL:2 * NL], sqA[:co, 0:NL], sqA[:co, NL:2 * NL], op=OP.add)
            nc.vector.tensor_tensor(nq[:co, NL:2 * NL], nq[:co, NL:2 * NL], sqA[:co, 2 * NL:3 * NL], op=OP.add)
            nc.gpsimd.tensor_tensor(nq[:co, 2 * NL:3 * NL], sqA[:co, 3 * NL:4 * NL], sqA[:co, 4 * NL:5 * NL], op=OP.add)
            nc.gpsimd.tensor_tensor(nq[:co, 2 * NL:3 * NL], nq[:co, 2 * NL:3 * NL], sqA[:co, 5 * NL:6 * NL], op=OP.add)
            nc.gpsimd.tensor_copy(nq[:co, 3 * NL:], sqA[:co, 6 * NL:7 * NL])
            nc.scalar.activation(nq[:co], nq[:co], AF.Sqrt, bias=SQ_EPS)
            for gr in range(4):
                nc.vector.tensor_scalar(nq[:co, gr * NL:(gr + 1) * NL],
                                        nq[:co, gr * NL:(gr + 1) * NL],
                                        colt[:, 9 + gr:10 + gr], colt[:, 13 + gr:14 + gr],
                                        op0=OP.mult, op1=OP.add)
            rec = wk.tile([128, 4 * NL], F32, tag="gt")
            scr = wk.tile([128, 4 * NL], F32, tag="qt")
            nc.vector.reciprocal_approx_accurate(rec[:co], nq[:co], scr[:co])
            for b in range(NB):
                nc.vector.tensor_tensor(bsl(xrt, b, P=co), rps[b],
                                        rec[:co, GRADES[b] * NL:(GRADES[b] + 1) * NL],
                                        op=OP.mult)

        # ---------- gp + combine with wl psum + bias ----------
        def gp_block(co, xs_t, xrt, lps, colt, blcol, xot):
            for j in range(NB):
                pb = wk.tile([128, NB * NL], F32, tag="big8")
                for i in range(NB):
                    kk = M2B[MASK[i] ^ MASK[j]]
                    eng = nc.vector
                    eng.scalar_tensor_tensor(bsl(pb, i, P=co), bsl(xs_t, i, P=co),
                                             colt[:, 18 + j * 8 + i:19 + j * 8 + i],
                                             bsl(xrt, kk, P=co), op0=OP.mult, op1=OP.mult)
                eng = nc.gpsimd
                eng.tensor_tensor(pb[:co, :4 * NL], pb[:co, :4 * NL], pb[:co, 4 * NL:], op=OP.add)
                eng.tensor_tensor(pb[:co, :2 * NL], pb[:co, :2 * NL], pb[:co, 2 * NL:4 * NL], op=OP.add)
                eng.tensor_tensor(pb[:co, :NL], pb[:co, :NL], pb[:co, NL:2 * NL], op=OP.add)
                bc = blcol[:, 0:1] if j == 0 else blcol[:, 1:2]
                nc.vector.scalar_tensor_tensor(bsl(xot, j, P=co), lps[j], bc,
                                               pb[:co, :NL], op0=OP.add, op1=OP.add)

        # ---------- LayerNorm channel-sum of norms ----------
        def ln_norm_sums(co, xot):
            sqo = wk.tile([128, NB * NL], F32, tag="big8")
            nc.scalar.activation(sqo[:co], xot[:co], AF.Square)
            nc.vector.tensor_tensor(sqo[:co, :4 * NL], sqo[:co, :4 * NL], sqo[:co, 4 * NL:], op=OP.add)
            nc.vector.tensor_tensor(sqo[:co, :2 * NL], sqo[:co, :2 * NL], sqo[:co, 2 * NL:4 * NL], op=OP.add)
            nc.vector.tensor_tensor(sqo[:co, :NL], sqo[:co, :NL], sqo[:co, NL:2 * NL], op=OP.add)
            nc.scalar.activation(sqo[:co, :NL], sqo[:co, :NL], AF.Sqrt, bias=2 * SQ_EPS)
            csum = wk.tile([128, NL], F32, tag="csum")
            nc.gpsimd.partition_all_reduce(csum[:co], sqo[:co, :NL], channels=co,
                                           reduce_op=bass_isa.ReduceOp.add)
            return csum

        def recip_row(row, co, nch):
            rr = wk.tile([128, NL], F32, tag="rr")
            nc.vector.tensor_scalar(rr[:co], row[:co], 1.0 / nch, RT2 * EPS,
                                    op0=OP.mult, op1=OP.add)
            rro = wk.tile([128, NL], F32, tag="rro")
            rrs = wk.tile([128, NL], F32, tag="rrs")
            nc.vector.reciprocal_approx_accurate(rro[:co], rr[:co], rrs[:co])
            return rro

        # ================= layers 1..3 =================
        x_cur = None
        for L in range(3):
            colt = lcol_t[L]
            lw_t = wst.tile([(1 if L == 0 else HID), 4 * HID], F32, tag="lww", bufs=2)
            nc.sync.dma_start(lw_t[:], lw_d[L].ap())
            lwlr_t = wst.tile([HID, 4 * 2 * HID], F32, tag="lwlr", bufs=2)
            nc.sync.dma_start(lwlr_t[:], lwlr_d[L].ap())
            yps = []
            for b in range(NB):
                gr = GRADES[b]
                pt = ps.tile([128, 512], F32, tag="ps")
                if L == 0:
                    x0b = wst.tile([1, NL], F32, tag="x0s", bufs=2)
                    nc.sync.dma_start(x0b[:], x0_d.ap()[b:b + 1, :])
                    rhs = x0b[:]
                else:
                    rhs = bsl(x_cur, b, P=HID)
                nc.tensor.matmul(pt[:HID, :NL], lhsT=lw_t[:, gr * HID:(gr + 1) * HID],
                                 rhs=rhs, start=True, stop=True)
                yps.append(pt[:HID, :NL])
            xs = big.tile([128, NB * NL], F32, tag="pk1")
            silu_block(HID, yps, colt, xs)
            if debug and L == 0:
                nc.sync.dma_start(dbg["d_x1"].ap(), xs[:HID])
            wps = []
            for b in range(NB):
                gr = GRADES[b]
                pt = ps.tile([128, 512], F32, tag="ps")
                nc.tensor.matmul(pt[:, :NL], lhsT=lwlr_t[:, gr * 128:(gr + 1) * 128],
                                 rhs=bsl(xs, b, P=HID), start=True, stop=True)
                wps.append(pt)
            lps = [p[0:HID, :NL] for p in wps]
            rps = [p[HID:2 * HID, :NL] for p in wps]
            xr = big.tile([128, NB * NL], F32, tag="pk2")
            normalize_block(HID, rps, colt, xr)
            xo = big.tile([128, NB * NL], F32, tag="pk3")
            gp_block(HID, xs, xr, lps, colt, colt[:, 82:84], xo)
            csum = ln_norm_sums(HID, xo)
            rro = recip_row(csum, HID, HID)
            xn = big.tile([128, NB * NL], MMD if L == 2 else F32, tag="pk4")
            for b in range(NB):
                nc.vector.scalar_tensor_tensor(bsl(xn, b, P=HID), bsl(xo, b, P=HID),
                                               colt[:, 17:18], rro[:HID, :NL],
                                               op0=OP.mult, op1=OP.mult)
            x_cur = xn
        x3r = x_cur
        if debug:
            nc.sync.dma_start(dbg["d_x3"].ap(), x3r[:HID].bitcast(F32))

        qb = cst.tile([128, NL], F32)
        nc.gpsimd.partition_broadcast(qb[:], qpos_t[:])

        # ================= layer 4 linear + silu (1024ch) =================
        x4s = []
        for ct in range(8):
            w4w = []
            for gr in range(4):
                wt = wst.tile([HID, 128], MMD, tag="w4w", bufs=4)
                nc.sync.dma_start(
                    wt[:], w4T_d.ap()[:, gr * 1024 + ct * 128:gr * 1024 + (ct + 1) * 128])
                w4w.append(wt)
            ypsl = []
            for b in range(NB):
                gr = GRADES[b]
                pt = ps.tile([128, 512], F32, tag="ps")
                nc.tensor.matmul(pt[:, :NL], lhsT=w4w[gr][:],
                                 rhs=bsl(x3r, b, P=HID), start=True, stop=True)
                ypsl.append(pt[:, :NL])
            xst = big.tile([128, NB * NL], MMD, tag=f"x4s{ct}")
            silu_block(128, ypsl, w4col_t[ct], xst)
            x4s.append(xst)

        ypsl = []
        for b in range(NB):
            gr = GRADES[b]
            pt = ps.tile([128, 512], F32, tag="ps")
            nc.tensor.matmul(pt[:, :NL], lhsT=w4myT_t[:, gr * CSH:(gr + 1) * CSH],
                             rhs=bsl(x3r, b, P=HID), start=True, stop=True)
            ypsl.append(pt[:, :NL])
        xmy = big.tile([128, NB * NL], F32, tag="pk1")
        silu_block(128, ypsl, l4col_t, xmy)
        if debug:
            nc.sync.dma_start(dbg["d_x4s"].ap(), xmy[:])

        # ================= layer 4 sgp (c-shard) =================
        def lin4(li):
            pts = [ps.tile([128, 512], F32, tag="ps", name=f"lin4_{li}_{b}")
                   for b in range(NB)]
            for kt in range(8):
                for gr in range(4):
                    wt = wst.tile([128, 128], MMD, tag="wt")
                    nc.sync.dma_start(wt[:], wlr4_d.ap()[li, gr, kt])
                    for b in BLADES_OF[gr]:
                        nc.tensor.matmul(pts[b][:, :NL], lhsT=wt[:],
                                         rhs=bsl(x4s[kt], b),
                                         start=(kt == 0), stop=(kt == 7))
            return pts

        rpts = lin4(1)
        xr4 = big.tile([128, NB * NL], F32, tag="pk2")
        normalize_block(128, [p[:, :NL] for p in rpts], l4col_t, xr4)
        if debug:
            nc.sync.dma_start(dbg["d_xr"].ap(), xr4[:])
        lpts = lin4(0)
        xo4 = big.tile([128, NB * NL], F32, tag="pk3")
        gp_block(128, xmy, xr4, [p[:, :NL] for p in lpts], l4col_t, bl4_t, xo4)
        if debug:
            nc.sync.dma_start(dbg["d_xo"].ap(), xo4[:])

        csum4 = ln_norm_sums(128, xo4)
        if no_collectives:
            sum_b = wk.tile([128, NL], F32, tag="sumt")
            nc.vector.tensor_scalar_mul(sum_b[:], csum4[:], 8.0)
        else:
            nc.sync.dma_start(ar_in.ap(), csum4[0:1, :])
            nc.gpsimd.collective_compute(
                "AllReduce", OP.add, replica_groups=[list(range(N_CORES))],
                ins=[ar_in.ap().opt()], outs=[ar_out.ap().opt()])
            sum_t = wk.tile([1, NL], F32, tag="sumt")
            nc.sync.dma_start(sum_t[:], ar_out.ap())
            sum_b = wk.tile([128, NL], F32, tag="csum")
            nc.gpsimd.partition_broadcast(sum_b[:], sum_t[:])
        if debug:
            nc.sync.dma_start(dbg["d_sum"].ap(), sum_b[0:1, :])
        rb4 = recip_row(sum_b, 128, 1024)

        Gt = big.tile([128, NB * NL], F32, tag="x4s0")
        for b in range(NB):
            nc.scalar.activation(bsl(Gt, b), qb[:], AF.Exp,
                                 scale=l4col_t[:, 82 + b:83 + b])
        for b in range(NB):
            eng = nc.vector if b % 2 == 0 else nc.gpsimd
            eng.tensor_tensor(bsl(Gt, b), bsl(Gt, b), rb4[:, :NL], op=OP.mult)
        for b in range(NB):
            nc.vector.scalar_tensor_tensor(bsl(xo4, b), bsl(xo4, b),
                                           l4col_t[:, 17:18], bsl(Gt, b),
                                           op0=OP.mult, op1=OP.mult)
        if debug:
            nc.sync.dma_start(dbg["d_T"].ap(), xo4[:])

        out_v = out_d.ap().rearrange("(ol l) (i m) n -> ol l i m n", l=NB, m=NB)
        for l in range(NB):
            for m in range(NB):
                ks = M2B[MASK[l] ^ MASK[m]]
                ot = ost.tile([128, NL], F32, tag="ot")
                eng = nc.vector if (l * 8 + m) % 2 == 0 else nc.gpsimd
                eng.tensor_scalar_mul(ot[:], bsl(xo4, ks),
                                      l4col_t[:, 90 + l * 8 + m:91 + l * 8 + m])
                nc.sync.dma_start(out_v[:, l, :, m, :], ot[:, :N])

    nc.compile()
    _PROG_CACHE[key] = nc
    return nc


def kernel(params):
    from concourse.bass_utils import run_bass_kernel_spmd
    in_maps = _host_prep(params)
    nc = _build_program()
    res = run_bass_kernel_spmd(nc, in_maps, core_ids=list(range(N_CORES)))
    out = np.concatenate([r["out"] for r in res.results], axis=0)
    return out.reshape(C_OUT * NB, C_IN * NB, K, K, K)
